# revision 16
# baseline (speedup 1.0000x reference)
"""Trainium2 Bass kernel for GQA attention (8-core SPMD, tensor-parallel heads).

Per-core shard c of 8 (4 q heads, 1 kv head):
  Projection (weights stationary, fp16): qT/kT/vT computed directly TRANSPOSED:
    psum[feat, tok] = WqkvT[d, feat].T @ xT[d, tok]. sm folded into Wk on host.
    No int8-quantization emulation (the reference's int8 round-trip is ~1%
    noise on the output; tolerance is 2e-2). fp8 was evaluated and rejected:
    for zero-mean data each fp8 matmul adds ~3-5% output rel err (no sqrt(K)
    averaging -- the output is itself a random walk of the same K terms).
  Attention: scoresT[t2, t1] = kT.T @ qT, two heads row-tiled concurrently
    (K=64 each, distinct row groups); two key-tiles packed gaplessly into one
    [128, 1024] 2-bank psum tile so each exp ACTIVATE covers ~1024 columns.
    Causal mask applied IN PSUM via an extra accumulate matmul of -30000
    (lhsT=identity, rhs=trineg) on diagonal tiles -> exp output is final;
    no DVE mask stage between exp and the AV matmuls.
    attT[hd, t1] = v_aug.T @ p with ones column -> row 64 = sumexp; normalize
    via one combined reciprocal [1, 1024] at partition 64 + K=1-matmul
    partition broadcast with the ones row AT partition 64 -- no cross-partition
    DMA anywhere in the normalize chain (the previous version's se0 DMA got
    stuck behind A2A-gated loads on the sync queue and stalled the PE ~10us).
    v is PE-transposed back to natural [tok, hd] layout two token-tiles per
    transpose (odd chunks moved to partitions 64:128 by a small DMA).
  Schedule: the attention inner loop is software-pipelined (ap's score MMs
    issue before ap-1's AV MMs) and projection ct-groups / o_proj chunks are
    interleaved as independent PE filler between ap iterations.
  o_proj (token-sharded): AllToAlls redistribute att [256 feat, tokens] ->
    [2048 feat, 128-token chunk per core]; each core holds FULL WoT and
    computes out[tok_chunk, :] = att_chunk.T @ Wo.T. Host stitches tokens.
    The second token group's A2A is split by head-pair; ork loads ride the
    GPSIMD DMA queue (not sync) so they never block the normalize a2a_in
    writes; even-k o_proj groups interleave ACROSS od blocks in the tail so
    only odd-k accumulations wait on the last half-A2A.
"""

import numpy as np
import ml_dtypes
from contextlib import ExitStack

import concourse.bass as bass
import concourse.mybir as mybir
import concourse.tile as tile
from concourse import bacc
from concourse.bass import ts, ds
from concourse.masks import make_identity

NCORES = 8
P = 128
S = 2048          # tokens
D = 2048          # model dim
HD = 64           # head dim
NHL = 4           # q heads per core
JQ = NHL * HD     # 256 (q feature rows per core)
NQKV = JQ + 2 * HD  # 384 wqkv columns per core (q0..q3, v, k)
TT = S // P       # 16 token tiles
DT = D // P       # 16 d tiles
NB = 4            # t1 blocks
BN = S // NB      # 512
TOK = 128         # a2a per-core token chunk
SM = HD ** -0.5   # 0.125 (folded into Wk on host)
NMASK = -30000.0  # causal mask additive constant (fits fp16)
F32 = mybir.dt.float32
BF16 = mybir.dt.bfloat16
FP16 = mybir.dt.float16
AF = mybir.ActivationFunctionType
ALU = mybir.AluOpType


def build_nc(debug_taps=False):
    nc = bacc.Bacc(target_bir_lowering=False, debug=False, num_devices=NCORES)
    xT = nc.declare_dram_parameter("xT", [D, S], FP16, isOutput=False)
    wqkv = nc.declare_dram_parameter("wqkv", [D, NQKV], FP16, isOutput=False)
    woT = nc.declare_dram_parameter("woT", [D, D], BF16, isOutput=False)
    trineg = nc.declare_dram_parameter("trineg", [P, P], FP16, isOutput=False)
    out_ext = nc.declare_dram_parameter("out", [2, P, D], F32, isOutput=True)

    taps = None
    if debug_taps:
        taps = {
            "qT_d": nc.declare_dram_parameter("qT_d", [P, 2, S], FP16, isOutput=True),
            "kT_d": nc.declare_dram_parameter("kT_d", [P, S], FP16, isOutput=True),
            "v_d": nc.declare_dram_parameter("v_d", [P, TT, HD + 1], BF16, isOutput=True),
            "a2a_in0_d": nc.declare_dram_parameter(
                "a2a_in0_d", [NCORES * JQ, TOK], BF16, isOutput=True),
        }
    with tile.TileContext(nc) as tc:
        with ExitStack() as ctx:
            _body(nc, tc, ctx, xT, wqkv, woT, trineg, out_ext, taps)
    nc.finalize()
    return nc


def _body(nc, tc, ctx, xT, wqkv, woT, trineg, out_ext, taps=None):
    # DRAM bounce buffers for the AllToAlls
    dram_pool = ctx.enter_context(tc.tile_pool(name="dram", bufs=1, space="DRAM"))
    a2a_in0 = dram_pool.tile([NCORES * JQ, TOK], BF16, name="a2a_in0", tag="ai0")
    a2a_out0 = dram_pool.tile([NCORES * JQ, TOK], BF16, name="a2a_out0", tag="ao0")
    a2a_in1 = [
        dram_pool.tile([NCORES * 2 * HD, TOK], BF16, name=f"a2a_in1{p}", tag=f"bi{p}")
        for p in range(2)
    ]
    a2a_out1 = [
        dram_pool.tile([NCORES * 2 * HD, TOK], BF16, name=f"a2a_out1{p}", tag=f"bo{p}")
        for p in range(2)
    ]
    singles = ctx.enter_context(tc.tile_pool(name="singles", bufs=1))
    xpool = ctx.enter_context(tc.tile_pool(name="xpool", bufs=2))
    vpool = ctx.enter_context(tc.tile_pool(name="vpool", bufs=2))
    pt_pool = ctx.enter_context(tc.tile_pool(name="pt", bufs=6))
    bc_sb = ctx.enter_context(tc.tile_pool(name="bc_sb", bufs=3))
    an_sb = ctx.enter_context(tc.tile_pool(name="an_sb", bufs=3))
    orhs = ctx.enter_context(tc.tile_pool(name="orhs", bufs=32))
    osb = ctx.enter_context(tc.tile_pool(name="osb", bufs=2))
    # PSUM: 8 banks of 2KB/partition total: 2 + 2*2 + 2 = 8
    ps_b = ctx.enter_context(tc.tile_pool(name="ps_b", bufs=2, space="PSUM"))
    ps_sc = ctx.enter_context(tc.tile_pool(name="ps_sc", bufs=2, space="PSUM"))
    ps_at = ctx.enter_context(tc.tile_pool(name="ps_at", bufs=2, space="PSUM"))

    # ---------------- persistent tiles ----------------
    wqkv_sb = singles.tile([P, DT, NQKV], FP16)
    _wsrc = wqkv.rearrange("(a p) n -> p a n", p=P)
    for c in range(2):
        nc.scalar.dma_start(out=wqkv_sb[:, 8 * c:8 * c + 8, :],
                            in_=_wsrc[:, 8 * c:8 * c + 8, :])
    woT_sb = singles.tile([P, DT, D], BF16)
    tri_sb = singles.tile([P, P], FP16)   # -30000 strictly above diagonal
    nc.scalar.dma_start(out=tri_sb, in_=trineg[:, :])
    id_fp16 = singles.tile([P, P], FP16)
    make_identity(nc, id_fp16)
    qT_sb = singles.tile([P, 2, S], FP16)   # [64*hh+hd, pair, t]
    kT_sb = singles.tile([P, S], FP16)      # sm-scaled k, duplicated halves
    v_sb = singles.tile([P, TT, HD + 1], BF16)
    nc.vector.memset(v_sb, 1.0)             # col 64 stays 1.0 (sumexp trick)
    ones_sb = singles.tile([HD + 1, HD], BF16)
    nc.vector.memset(ones_sb, 1.0)

    def xb_load(tb):
        xb = xpool.tile([P, DT, BN], FP16, tag="xb")
        xsrc = xT[:, ts(tb, BN)].rearrange("(a p) m -> p a m", p=P)
        for c in range(2):
            nc.sync.dma_start(out=xb[:, 8 * c:8 * c + 8, :],
                              in_=xsrc[:, 8 * c:8 * c + 8, :])
        return xb

    def proj_ct(tb, xb, ct):
        """One 16-MM projection group (PE filler)."""
        ps = ps_b.tile([P, BN], F32, tag="mm")
        for d in range(DT):
            nc.tensor.matmul(
                ps, lhsT=wqkv_sb[:, d, ts(ct, P)], rhs=xb[:, d, :],
                start=(d == 0), stop=(d == DT - 1))
        if ct < 2:
            nc.vector.tensor_copy(qT_sb[:, ct, ts(tb, BN)], ps)
            return None
        # cols 256:320 = v feats (psum partitions 0:64),
        # cols 320:384 = k feats (partitions 64:128)
        nc.vector.tensor_copy(kT_sb[HD:P, ts(tb, BN)], ps[HD:P, :])
        nc.sync.dma_start(out=kT_sb[0:HD, ts(tb, BN)],
                          in_=kT_sb[HD:P, ts(tb, BN)])
        vtmp = vpool.tile([HD, BN], FP16, tag="vt")
        nc.vector.tensor_copy(vtmp, ps[0:HD, :])
        return vtmp

    def v_trans(tb, vtmp):
        """Transpose vT -> natural v, two token-tiles per PE op (PE filler).
        vst partitions 0:64 = even chunks (DVE copy), 64:128 = odd chunks
        (small cross-partition DMA)."""
        vst = vpool.tile([P, 2, P], FP16, tag="vst")
        vsrc = vtmp.rearrange("p (a m) -> p a m", m=2 * P)
        nc.vector.tensor_copy(vst[0:HD, :, :], vsrc[:, :, 0:P])
        nc.sync.dma_start(out=vst[HD:P, :, :], in_=vsrc[:, :, P:2 * P])
        for g in range(2):
            vt = ps_b.tile([P, P], FP16, tag="mm")
            nc.tensor.transpose(vt, vst[:, g, :], id_fp16)
            nc.vector.tensor_copy(v_sb[:, 4 * tb + 2 * g, 0:HD], vt[:, 0:HD])
            nc.vector.tensor_copy(v_sb[:, 4 * tb + 2 * g + 1, 0:HD], vt[:, HD:P])

    def proj_fillers(tb):
        xb = xb_load(tb)
        st = {}

        def f_ct(ct):
            def f():
                r = proj_ct(tb, xb, ct)
                if ct == 2:
                    st["vtmp"] = r
            return f
        # k/v group first: attention block b's pair-0 scores need kT before
        # the q pairs, and the kT-dup DMA should land as early as possible
        return [f_ct(2), f_ct(0), f_ct(1), lambda: v_trans(tb, st["vtmp"])]

    def oproj_chunk(g, od, ork, korder=None, pso=None, finish=True):
        """o_proj accumulation group for output-dim block od (PE filler).
        korder restricts/reorders the k-tiles; pass pso + finish=False to
        accumulate in two phases (even k-tiles before the odd half-A2A lands)."""
        ks = list(korder) if korder is not None else list(range(DT))
        if pso is None:
            pso = ps_b.tile([P, BN], F32, tag="mm", name=f"op{g}_{od}")
        first = not finish or len(ks) == DT
        for i, k in enumerate(ks):
            nc.tensor.matmul(
                pso, lhsT=ork[k], rhs=woT_sb[:, k, ds(od * BN, BN)],
                start=(first and i == 0), stop=(finish and i == len(ks) - 1))
        if not finish:
            return pso
        ot = osb.tile([P, BN], F32, tag="ot", name=f"ot{g}_{od}")
        nc.vector.tensor_copy(ot, pso)
        nc.sync.dma_start(out=out_ext[g, :, ds(od * BN, BN)], in_=ot)
        return None

    def trigger_a2a(src_t, dst_t):
        nc.gpsimd.collective_compute(
            "AllToAll", ALU.bypass,
            replica_groups=[list(range(NCORES))],
            ins=[src_t[:, :]],
            outs=[dst_t[:, :]])

    def att_block(b, fillers, pair_end=None):
        """Attention for query block b, software-pipelined, sprinkling
        `fillers` (independent PE work) between ap iterations."""
        na = 4 * (b + 1)
        g = b // 2
        fillers = list(fillers)
        stride = max(1, na // max(1, len(fillers)))
        slot = 0

        def do_norm(pair, atp):
            # normalize by sumexp (row 64 of atp), emit bf16 into the a2a
            # input buffer. Both heads' sumexp rows are gathered at partition
            # 64, hop to partition 0 with ONE small DMA (sync queue -- safe
            # now that no A2A-gated DMA rides sync), then one combined
            # reciprocal + two K=1 broadcast matmuls from partition 0.
            se2 = bc_sb.tile([HD + 1, 2, BN], F32, tag="se")
            for hh in range(2):
                nc.vector.tensor_copy(se2[HD:HD + 1, hh, :], atp[hh][HD:HD + 1, :])
            nc.sync.dma_start(out=se2[0:1, :, :], in_=se2[HD:HD + 1, :, :])
            rcp = bc_sb.tile([HD + 1, 2, BN], F32, tag="rcp")
            nc.vector.reciprocal_approx_fast(rcp[0:1, :, :], se2[0:1, :, :])
            rcpb = bc_sb.tile([HD + 1, 2, BN], BF16, tag="rcpb")
            nc.vector.tensor_copy(rcpb[0:1, :, :], rcp[0:1, :, :])
            for hh in range(2):
                rbs = ps_b.tile([HD, BN], F32, tag="mm",
                                name=f"rbs{b}_{pair}_{hh}")
                nc.tensor.matmul(
                    rbs, lhsT=ones_sb[0:1, 0:HD],
                    rhs=rcpb[0:1, hh, :], start=True, stop=True)
                rbs_sb = bc_sb.tile([HD, BN], F32, tag="rbs_sb")
                nc.vector.tensor_copy(rbs_sb, rbs)
                ans = an_sb.tile([HD, BN], BF16, tag="ans")
                nc.vector.tensor_mul(ans, atp[hh][0:HD, :], rbs_sb)
                # one batched DMA scatters all 4 destination-core chunks
                src = ans.rearrange("p (a c) -> p a c", a=4)
                if g == 0:
                    dstv = a2a_in0.rearrange("(j p) c -> p j c", p=JQ)
                    dst = dstv[ds(HD * (2 * pair + hh), HD),
                               ds(4 * (b % 2), 4), :]
                else:
                    dstv = a2a_in1[pair].rearrange("(j p) c -> p j c", p=2 * HD)
                    dst = dstv[ds(HD * hh, HD), ds(4 * (b % 2), 4), :]
                nc.sync.dma_start(out=dst, in_=src)
            if pair_end is not None:
                pair_end(pair)

        pending_norm = None
        for pair in range(2):
            atp = [ps_at.tile([HD + 1, BN], F32, tag="at",
                              name=f"at{b}_{pair}_{hh}") for hh in range(2)]
            pend = None
            for ap in range(na // 2):
                a0, a1 = 2 * ap, 2 * ap + 1
                off0 = max(0, a0 - 4 * b) * P
                off1 = max(0, a1 - 4 * b) * P
                len1 = BN - off1
                # 4 score MMs, hh-alternated so row groups 0/1 overlap; causal
                # mask accumulated in-psum on diagonal tiles
                scs = []
                for hh in range(2):
                    scs.append(ps_sc.tile([P, 2 * BN], F32, tag="sc",
                                          name=f"sc{b}_{pair}_{ap}_{hh}"))
                d0 = a0 >= 4 * b
                d1 = a1 >= 4 * b
                for hh in range(2):
                    rows = slice(HD * hh, HD * hh + HD)
                    nc.tensor.matmul(
                        scs[hh][:, off0:BN], lhsT=kT_sb[rows, ts(a0, P)],
                        rhs=qT_sb[rows, pair, ds(b * BN + off0, BN - off0)],
                        start=True, stop=not d0)
                if d0:
                    for hh in range(2):
                        nc.tensor.matmul(
                            scs[hh][:, off0:off0 + P], lhsT=id_fp16,
                            rhs=tri_sb, start=False, stop=True)
                for hh in range(2):
                    rows = slice(HD * hh, HD * hh + HD)
                    nc.tensor.matmul(
                        scs[hh][:, BN:BN + len1], lhsT=kT_sb[rows, ts(a1, P)],
                        rhs=qT_sb[rows, pair, ds(b * BN + off1, len1)],
                        start=True, stop=not d1)
                if d1:
                    for hh in range(2):
                        nc.tensor.matmul(
                            scs[hh][:, BN:BN + P], lhsT=id_fp16,
                            rhs=tri_sb, start=False, stop=True)
                pts = []
                for hh in range(2):
                    pt = pt_pool.tile([P, 2 * BN], BF16, tag="pt",
                                      name=f"pt{b}_{pair}_{ap}_{hh}")
                    nc.scalar.activation(
                        out=pt[:, off0:BN + len1], in_=scs[hh][:, off0:BN + len1],
                        func=AF.Exp)
                    pts.append(pt)
                # deferred previous-pair normalize: now the PE has this pair's
                # score MMs in front of it, so the K=1 broadcast matmul's wait
                # on the reciprocal chain is hidden
                if pending_norm is not None and ap == 1:
                    pending_norm()
                    pending_norm = None
                # AV matmuls of the previous ap (software pipelining)
                if pend is not None:
                    _issue_avs(atp, pend, na)
                pend = (a0, a1, off0, off1, len1, pts)
                slot += 1
                if fillers and slot % stride == 0:
                    fillers.pop(0)()
            _issue_avs(atp, pend, na)
            if pair == 0:
                pending_norm = (lambda atp=atp: do_norm(0, atp))
            else:
                do_norm(1, atp)
        for f in fillers:
            f()

    def _issue_avs(atp, pend, na):
        a0, a1, off0, off1, len1, pts = pend
        for hh in range(2):
            nc.tensor.matmul(
                atp[hh][:, off0:], lhsT=v_sb[:, a0, :],
                rhs=pts[hh][:, off0:BN], start=(a0 == 0), stop=False)
            nc.tensor.matmul(
                atp[hh][:, off1:], lhsT=v_sb[:, a1, :],
                rhs=pts[hh][:, BN:BN + len1], start=False, stop=(a1 == na - 1))

    # ---------------- schedule ----------------
    for f in proj_fillers(0):
        f()
    att_block(0, proj_fillers(1))
    # prefetch full WoT during the attention phase (gpsimd DMA queue; the
    # only other things on gpsimd are the collective triggers and ork loads)
    _wosrc = woT.rearrange("(a p) n -> p a n", p=P)
    for c in range(DT):
        nc.gpsimd.dma_start(out=woT_sb[:, c:c + 1, :], in_=_wosrc[:, c:c + 1, :])
    att_block(1, proj_fillers(2))
    trigger_a2a(a2a_in0, a2a_out0)
    att_block(2, proj_fillers(3))
    # ork0 loads on the GPSIMD queue (NOT sync): they wait on the A2A#0
    # semaphore and must not sit in front of later normalize a2a_in writes
    # on the sync queue.
    ork0 = []
    a2a0_r = a2a_out0.rearrange("(a p) m -> p a m", p=P)
    for k in range(DT):
        rt = orhs.tile([P, TOK], BF16, tag="rt", name=f"rt0_{k}")
        nc.gpsimd.dma_start(out=rt, in_=a2a0_r[:, k, :])
        ork0.append(rt)
    # ork1: even k-tiles come from the pair-0 half A2A, odd from pair-1
    ork1 = [None] * DT

    def b3_pair_end(pair):
        trigger_a2a(a2a_in1[pair], a2a_out1[pair])
        a2a1_r = a2a_out1[pair].rearrange("(a p) m -> p a m", p=P)
        for s in range(NCORES):
            rt = orhs.tile([P, TOK], BF16, tag="rt", name=f"rt1_{2 * s + pair}")
            nc.gpsimd.dma_start(out=rt, in_=a2a1_r[:, s, :])
            ork1[2 * s + pair] = rt

    att_block(3, [lambda: oproj_chunk(0, 0, ork0), lambda: oproj_chunk(0, 1, ork0)],
              pair_end=b3_pair_end)
    # tail: even-k g1 groups (data from A2A#1a) interleave ACROSS od blocks
    # while A2A#1b is in flight; only odd-k accumulations wait on it. ps_b has
    # 2 bufs -> at most two open o_proj-1 psum groups at a time.
    evens = [0, 2, 4, 6, 8, 10, 12, 14]
    odds = [1, 3, 5, 7, 9, 11, 13, 15]
    oproj_chunk(0, 2, ork0)
    pso0 = oproj_chunk(1, 0, ork1, korder=evens, finish=False)
    oproj_chunk(0, 3, ork0)
    pso1 = oproj_chunk(1, 1, ork1, korder=evens, finish=False)
    if taps is not None:
        nc.sync.dma_start(out=taps["qT_d"][:, :, :], in_=qT_sb)
        nc.sync.dma_start(out=taps["kT_d"][:, :], in_=kT_sb)
        nc.sync.dma_start(out=taps["v_d"][:, :, :], in_=v_sb)
        nc.sync.dma_start(out=taps["a2a_in0_d"][:, :], in_=a2a_in0[:, :])
    oproj_chunk(1, 0, ork1, korder=odds, pso=pso0)
    pso2 = oproj_chunk(1, 2, ork1, korder=evens, finish=False)
    oproj_chunk(1, 1, ork1, korder=odds, pso=pso1)
    pso3 = oproj_chunk(1, 3, ork1, korder=evens, finish=False)
    oproj_chunk(1, 2, ork1, korder=odds, pso=pso2)
    oproj_chunk(1, 3, ork1, korder=odds, pso=pso3)


# ---------------- host side ----------------

def prep_in_maps(x, Wq, Wk, Wv, Wo):
    bf = ml_dtypes.bfloat16
    xTh = np.ascontiguousarray(x.reshape(S, D).T.astype(np.float16))
    trineg = np.where(np.arange(P)[:, None] <= np.arange(P)[None, :],
                      np.float16(0.0), np.float16(NMASK))
    trineg = np.ascontiguousarray(trineg.astype(np.float16))
    woT_h = np.ascontiguousarray(Wo.T.astype(bf))
    in_maps = []
    for c in range(NCORES):
        wq = Wq[c * JQ:(c + 1) * JQ, :].T
        wk = Wk[c * HD:(c + 1) * HD, :].T * SM
        wv = Wv[c * HD:(c + 1) * HD, :].T
        wqkv_h = np.ascontiguousarray(
            np.concatenate([wq, wv, wk], axis=1).astype(np.float16))
        in_maps.append({"xT": xTh, "wqkv": wqkv_h, "woT": woT_h,
                        "trineg": trineg})
    return in_maps


def unshard(results):
    out = np.empty((S, D), dtype=np.float32)
    for c in range(NCORES):
        o = np.asarray(results[c]["out"]).reshape(2, TOK, D)
        out[c * TOK:(c + 1) * TOK, :] = o[0]
        out[S // 2 + c * TOK:S // 2 + (c + 1) * TOK, :] = o[1]
    return out.reshape(1, S, D)


def kernel(x, Wq, Wk, Wv, Wo):
    from concourse.bass_utils import run_bass_kernel_spmd
    nc = build_nc()
    in_maps = prep_in_maps(x, Wq, Wk, Wv, Wo)
    res = run_bass_kernel_spmd(nc, in_maps, core_ids=list(range(NCORES)))
    return unshard(res.results)


# revision 24
# speedup vs baseline: 1.0260x; 1.0260x over previous
"""Trainium2 Bass kernel for GQA attention (8-core SPMD, tensor-parallel heads).

Per-core shard c of 8 (4 q heads, 1 kv head):
  Projection (weights stationary, fp16): qT/kT/vT computed directly TRANSPOSED:
    psum[feat, tok] = WqkvT[d, feat].T @ xT[d, tok]. sm folded into Wk on host.
    No int8-quantization emulation (the reference's int8 round-trip is ~1%
    noise on the output; tolerance is 2e-2). fp8 was evaluated and rejected:
    for zero-mean data each fp8 matmul adds ~3-5% output rel err (no sqrt(K)
    averaging -- the output is itself a random walk of the same K terms).
  Attention: scoresT[t2, t1] = kT.T @ qT, two heads row-tiled concurrently
    (K=64 each, distinct row groups); two key-tiles packed gaplessly into one
    [128, 1024] 2-bank psum tile so each exp ACTIVATE covers ~1024 columns.
    Causal mask applied IN PSUM via an extra accumulate matmul of -30000
    (lhsT=identity, rhs=trineg) on diagonal tiles -> exp output is final;
    no DVE mask stage between exp and the AV matmuls.
    attT[hd, t1] = v_aug.T @ p with ones column -> row 64 = sumexp; normalize
    via one combined reciprocal [1, 1024] at partition 64 + K=1-matmul
    partition broadcast with the ones row AT partition 64 -- no cross-partition
    DMA anywhere in the normalize chain (the previous version's se0 DMA got
    stuck behind A2A-gated loads on the sync queue and stalled the PE ~10us).
    v is PE-transposed back to natural [tok, hd] layout two token-tiles per
    transpose (odd chunks moved to partitions 64:128 by a small DMA).
  Schedule: the attention inner loop is software-pipelined (ap's score MMs
    issue before ap-1's AV MMs) and projection ct-groups / o_proj chunks are
    interleaved as independent PE filler between ap iterations.
  o_proj (token-sharded): AllToAlls redistribute att [256 feat, tokens] ->
    [2048 feat, 128-token chunk per core]; each core holds FULL WoT and
    computes out[tok_chunk, :] = att_chunk.T @ Wo.T. Host stitches tokens.
    The second token group's A2A is split by head-pair; ork loads ride the
    GPSIMD DMA queue (not sync) so they never block the normalize a2a_in
    writes; even-k o_proj groups interleave ACROSS od blocks in the tail so
    only odd-k accumulations wait on the last half-A2A.
"""

import numpy as np
import ml_dtypes
from contextlib import ExitStack

import concourse.bass as bass
import concourse.mybir as mybir
import concourse.tile as tile
from concourse import bacc
from concourse.bass import ts, ds
from concourse.masks import make_identity

NCORES = 8
P = 128
S = 2048          # tokens
D = 2048          # model dim
HD = 64           # head dim
NHL = 4           # q heads per core
JQ = NHL * HD     # 256 (q feature rows per core)
NQKV = JQ + 2 * HD  # 384 wqkv columns per core (q0..q3, v, k)
TT = S // P       # 16 token tiles
DT = D // P       # 16 d tiles
NB = 4            # t1 blocks
BN = S // NB      # 512
TOK = 128         # a2a per-core token chunk
SM = HD ** -0.5   # 0.125 (folded into Wk on host)
NMASK = -30000.0  # causal mask additive constant (fits fp16)
F32 = mybir.dt.float32
BF16 = mybir.dt.bfloat16
FP16 = mybir.dt.float16
AF = mybir.ActivationFunctionType
ALU = mybir.AluOpType


def build_nc(debug_taps=False):
    nc = bacc.Bacc(target_bir_lowering=False, debug=False, num_devices=NCORES)
    xT = nc.declare_dram_parameter("xT", [D, S], FP16, isOutput=False)
    wqkv = nc.declare_dram_parameter("wqkv", [D, NQKV], FP16, isOutput=False)
    woT = nc.declare_dram_parameter("woT", [D, D], BF16, isOutput=False)
    trineg = nc.declare_dram_parameter("trineg", [P, P], FP16, isOutput=False)
    out_ext = nc.declare_dram_parameter("out", [2, P, D], F32, isOutput=True)

    taps = None
    if debug_taps:
        taps = {
            "qT_d": nc.declare_dram_parameter("qT_d", [P, 2, S], FP16, isOutput=True),
            "kT_d": nc.declare_dram_parameter("kT_d", [P, S], FP16, isOutput=True),
            "v_d": nc.declare_dram_parameter("v_d", [P, TT, HD + 1], BF16, isOutput=True),
            "a2a_in0_d": nc.declare_dram_parameter(
                "a2a_in0_d", [NCORES * JQ, TOK], BF16, isOutput=True),
        }
    with tile.TileContext(nc) as tc:
        with ExitStack() as ctx:
            _body(nc, tc, ctx, xT, wqkv, woT, trineg, out_ext, taps)
    nc.finalize()
    return nc


def _body(nc, tc, ctx, xT, wqkv, woT, trineg, out_ext, taps=None):
    # DRAM bounce buffers for the AllToAlls
    dram_pool = ctx.enter_context(tc.tile_pool(name="dram", bufs=1, space="DRAM"))
    a2a_in0 = dram_pool.tile([NCORES * JQ, TOK], BF16, name="a2a_in0", tag="ai0")
    a2a_out0 = dram_pool.tile([NCORES * JQ, TOK], BF16, name="a2a_out0", tag="ao0")
    a2a_in1 = [
        dram_pool.tile([NCORES * 2 * HD, TOK], BF16, name=f"a2a_in1{p}", tag=f"bi{p}")
        for p in range(2)
    ]
    a2a_out1 = [
        dram_pool.tile([NCORES * 2 * HD, TOK], BF16, name=f"a2a_out1{p}", tag=f"bo{p}")
        for p in range(2)
    ]
    singles = ctx.enter_context(tc.tile_pool(name="singles", bufs=1))
    xpool = ctx.enter_context(tc.tile_pool(name="xpool", bufs=2))
    vpool = ctx.enter_context(tc.tile_pool(name="vpool", bufs=2))
    pt_pool = ctx.enter_context(tc.tile_pool(name="pt", bufs=6))
    bc_sb = ctx.enter_context(tc.tile_pool(name="bc_sb", bufs=3))
    an_sb = ctx.enter_context(tc.tile_pool(name="an_sb", bufs=3))
    orhs = ctx.enter_context(tc.tile_pool(name="orhs", bufs=32))
    osb = ctx.enter_context(tc.tile_pool(name="osb", bufs=2))
    # PSUM: 8 banks of 2KB/partition total: 2 + 2*2 + 2 = 8
    ps_b = ctx.enter_context(tc.tile_pool(name="ps_b", bufs=2, space="PSUM"))
    ps_sc = ctx.enter_context(tc.tile_pool(name="ps_sc", bufs=2, space="PSUM"))
    ps_at = ctx.enter_context(tc.tile_pool(name="ps_at", bufs=2, space="PSUM"))

    # ---------------- persistent tiles ----------------
    wqkv_sb = singles.tile([P, DT, NQKV], FP16)
    _wsrc = wqkv.rearrange("(a p) n -> p a n", p=P)
    # many small DMAs fan out across the 16 DMA engines (a consolidated DMA
    # serializes on one engine and delays the first projection matmul)
    for c in range(8):
        nc.scalar.dma_start(out=wqkv_sb[:, 2 * c:2 * c + 2, :],
                            in_=_wsrc[:, 2 * c:2 * c + 2, :])
    woT_sb = singles.tile([P, DT, D], BF16)
    tri_sb = singles.tile([P, P], FP16)   # -30000 strictly above diagonal
    nc.scalar.dma_start(out=tri_sb, in_=trineg[:, :])
    id_fp16 = singles.tile([P, P], FP16)
    make_identity(nc, id_fp16)
    qT_sb = singles.tile([P, 2, S], FP16)   # [64*hh+hd, pair, t]
    kT_sb = singles.tile([P, S], FP16)      # sm-scaled k, duplicated halves
    v_sb = singles.tile([P, TT, HD + 1], BF16)
    nc.vector.memset(v_sb, 1.0)             # col 64 stays 1.0 (sumexp trick)
    ones_sb = singles.tile([HD + 1, HD], BF16)
    nc.vector.memset(ones_sb, 1.0)

    def xb_load(tb):
        xb = xpool.tile([P, DT, BN], FP16, tag="xb")
        xsrc = xT[:, ts(tb, BN)].rearrange("(a p) m -> p a m", p=P)
        for c in range(4):
            nc.sync.dma_start(out=xb[:, 4 * c:4 * c + 4, :],
                              in_=xsrc[:, 4 * c:4 * c + 4, :])
        return xb

    def proj_ct(tb, xb, ct):
        """One 16-MM projection group (PE filler)."""
        ps = ps_b.tile([P, BN], F32, tag="mm")
        for d in range(DT):
            nc.tensor.matmul(
                ps, lhsT=wqkv_sb[:, d, ts(ct, P)], rhs=xb[:, d, :],
                start=(d == 0), stop=(d == DT - 1))
        if ct < 2:
            nc.vector.tensor_copy(qT_sb[:, ct, ts(tb, BN)], ps)
            return None
        # cols 256:320 = v feats (psum partitions 0:64),
        # cols 320:384 = k feats (partitions 64:128)
        nc.vector.tensor_copy(kT_sb[HD:P, ts(tb, BN)], ps[HD:P, :])
        nc.sync.dma_start(out=kT_sb[0:HD, ts(tb, BN)],
                          in_=kT_sb[HD:P, ts(tb, BN)])
        vtmp = vpool.tile([HD, BN], FP16, tag="vt")
        nc.vector.tensor_copy(vtmp, ps[0:HD, :])
        return vtmp

    def v_trans(tb, vtmp):
        """Transpose vT -> natural v, two token-tiles per PE op (PE filler).
        vst partitions 0:64 = even chunks (DVE copy), 64:128 = odd chunks
        (small cross-partition DMA)."""
        vst = vpool.tile([P, 2, P], FP16, tag="vst")
        vsrc = vtmp.rearrange("p (a m) -> p a m", m=2 * P)
        nc.vector.tensor_copy(vst[0:HD, :, :], vsrc[:, :, 0:P])
        nc.sync.dma_start(out=vst[HD:P, :, :], in_=vsrc[:, :, P:2 * P])
        for g in range(2):
            vt = ps_b.tile([P, P], FP16, tag="mm")
            nc.tensor.transpose(vt, vst[:, g, :], id_fp16)
            nc.vector.tensor_copy(v_sb[:, 4 * tb + 2 * g, 0:HD], vt[:, 0:HD])
            nc.vector.tensor_copy(v_sb[:, 4 * tb + 2 * g + 1, 0:HD], vt[:, HD:P])

    def proj_fillers(tb):
        xb = xb_load(tb)
        st = {}

        def f_ct(ct):
            def f():
                r = proj_ct(tb, xb, ct)
                if ct == 2:
                    st["vtmp"] = r
            return f
        # k/v group first: attention block b's pair-0 scores need kT before
        # the q pairs, and the kT-dup DMA should land as early as possible
        return [f_ct(2), f_ct(0), f_ct(1), lambda: v_trans(tb, st["vtmp"])]

    def oproj_chunk(g, od, ork, korder=None, pso=None, finish=True):
        """o_proj accumulation group for output-dim block od (PE filler).
        korder restricts/reorders the k-tiles; pass pso + finish=False to
        accumulate in two phases (even k-tiles before the odd half-A2A lands)."""
        ks = list(korder) if korder is not None else list(range(DT))
        if pso is None:
            pso = ps_b.tile([P, BN], F32, tag="mm", name=f"op{g}_{od}")
        first = not finish or len(ks) == DT
        for i, k in enumerate(ks):
            nc.tensor.matmul(
                pso, lhsT=ork[k], rhs=woT_sb[:, k, ds(od * BN, BN)],
                start=(first and i == 0), stop=(finish and i == len(ks) - 1))
        if not finish:
            return pso
        ot = osb.tile([P, BN], F32, tag="ot", name=f"ot{g}_{od}")
        nc.vector.tensor_copy(ot, pso)
        # scalar queue: idle at the tail, and on sync these would queue
        # behind the #1b-gated odd ork loads
        nc.scalar.dma_start(out=out_ext[g, :, ds(od * BN, BN)], in_=ot)
        return None

    def trigger_a2a(src_t, dst_t):
        nc.gpsimd.collective_compute(
            "AllToAll", ALU.bypass,
            replica_groups=[list(range(NCORES))],
            ins=[src_t[:, :]],
            outs=[dst_t[:, :]])

    def att_block(b, fillers, pair_end=None):
        """Attention for query block b, software-pipelined, sprinkling
        `fillers` (independent PE work) between ap iterations."""
        na = 4 * (b + 1)
        g = b // 2
        fillers = list(fillers)
        stride = max(1, na // max(1, len(fillers)))
        slot = 0

        def do_norm(pair, atp):
            # normalize by sumexp (row 64 of atp), emit bf16 into the a2a
            # input buffer. Both heads' sumexp rows are gathered at partition
            # 64, hop to partition 0 with ONE small DMA (sync queue -- safe
            # now that no A2A-gated DMA rides sync), then one combined
            # reciprocal + two K=1 broadcast matmuls from partition 0.
            # copy sumexp rows to SBUF, hop to partition 0 with one small DMA,
            # THEN reciprocal at partition 0. Two HW/sim divergences live
            # here: custom-DVE ops (reciprocal) return junk on HW when
            # reading PSUM directly OR when based at partition 64 -- both
            # pass CoreSim. Keep the chain exactly in this proven shape.
            se2 = bc_sb.tile([HD + 1, 2, BN], F32, tag="se")
            for hh in range(2):
                nc.vector.tensor_copy(se2[HD:HD + 1, hh, :], atp[hh][HD:HD + 1, :])
            nc.sync.dma_start(out=se2[0:1, :, :], in_=se2[HD:HD + 1, :, :])
            rcp = bc_sb.tile([HD + 1, 2, BN], F32, tag="rcp")
            nc.vector.reciprocal_approx_fast(rcp[0:1, :, :], se2[0:1, :, :])
            rcpb = bc_sb.tile([HD + 1, 2, BN], BF16, tag="rcpb")
            nc.vector.tensor_copy(rcpb[0:1, :, :], rcp[0:1, :, :])
            for hh in range(2):
                rbs = ps_b.tile([HD, BN], F32, tag="mm",
                                name=f"rbs{b}_{pair}_{hh}")
                nc.tensor.matmul(
                    rbs, lhsT=ones_sb[0:1, 0:HD],
                    rhs=rcpb[0:1, hh, :], start=True, stop=True)
                rbs_sb = bc_sb.tile([HD, BN], F32, tag="rbs_sb")
                nc.vector.tensor_copy(rbs_sb, rbs)
                ans = an_sb.tile([HD, BN], BF16, tag="ans")
                nc.vector.tensor_mul(ans, atp[hh][0:HD, :], rbs_sb)
                # one batched DMA scatters all 4 destination-core chunks
                src = ans.rearrange("p (a c) -> p a c", a=4)
                if g == 0:
                    dstv = a2a_in0.rearrange("(j p) c -> p j c", p=JQ)
                    dst = dstv[ds(HD * (2 * pair + hh), HD),
                               ds(4 * (b % 2), 4), :]
                else:
                    dstv = a2a_in1[pair].rearrange("(j p) c -> p j c", p=2 * HD)
                    dst = dstv[ds(HD * hh, HD), ds(4 * (b % 2), 4), :]
                nc.sync.dma_start(out=dst, in_=src)
            if pair_end is not None:
                pair_end(pair)

        pending_norm = None
        for pair in range(2):
            atp = [ps_at.tile([HD + 1, BN], F32, tag="at",
                              name=f"at{b}_{pair}_{hh}") for hh in range(2)]
            pend = None
            for ap in range(na // 2):
                a0, a1 = 2 * ap, 2 * ap + 1
                off0 = max(0, a0 - 4 * b) * P
                off1 = max(0, a1 - 4 * b) * P
                len1 = BN - off1
                # 4 score MMs, hh-alternated so row groups 0/1 overlap; causal
                # mask accumulated in-psum on diagonal tiles
                scs = []
                for hh in range(2):
                    scs.append(ps_sc.tile([P, 2 * BN], F32, tag="sc",
                                          name=f"sc{b}_{pair}_{ap}_{hh}"))
                d0 = a0 >= 4 * b
                d1 = a1 >= 4 * b
                for hh in range(2):
                    rows = slice(HD * hh, HD * hh + HD)
                    nc.tensor.matmul(
                        scs[hh][:, off0:BN], lhsT=kT_sb[rows, ts(a0, P)],
                        rhs=qT_sb[rows, pair, ds(b * BN + off0, BN - off0)],
                        start=True, stop=not d0)
                if d0:
                    for hh in range(2):
                        nc.tensor.matmul(
                            scs[hh][:, off0:off0 + P], lhsT=id_fp16,
                            rhs=tri_sb, start=False, stop=True)
                for hh in range(2):
                    rows = slice(HD * hh, HD * hh + HD)
                    nc.tensor.matmul(
                        scs[hh][:, BN:BN + len1], lhsT=kT_sb[rows, ts(a1, P)],
                        rhs=qT_sb[rows, pair, ds(b * BN + off1, len1)],
                        start=True, stop=not d1)
                if d1:
                    for hh in range(2):
                        nc.tensor.matmul(
                            scs[hh][:, BN:BN + P], lhsT=id_fp16,
                            rhs=tri_sb, start=False, stop=True)
                pts = []
                for hh in range(2):
                    pt = pt_pool.tile([P, 2 * BN], BF16, tag="pt",
                                      name=f"pt{b}_{pair}_{ap}_{hh}")
                    nc.scalar.activation(
                        out=pt[:, off0:BN + len1], in_=scs[hh][:, off0:BN + len1],
                        func=AF.Exp)
                    pts.append(pt)
                # deferred previous-pair normalize: now the PE has this pair's
                # score MMs in front of it, so the K=1 broadcast matmul's wait
                # on the reciprocal chain is hidden. Two aps of runway (where
                # available) -- one was not enough for the DVE chain and the
                # resulting ~1.1us PE stall re-throttled HAM at every pair
                # boundary.
                if pending_norm is not None and ap == min(2, na // 2 - 1):
                    pending_norm()
                    pending_norm = None
                # AV matmuls of the previous ap (software pipelining)
                if pend is not None:
                    _issue_avs(atp, pend, na)
                pend = (a0, a1, off0, off1, len1, pts)
                slot += 1
                if fillers and slot % stride == 0:
                    fillers.pop(0)()
            _issue_avs(atp, pend, na)
            if pair == 0:
                pending_norm = (lambda atp=atp: do_norm(0, atp))
            else:
                do_norm(1, atp)
        for f in fillers:
            f()

    def _issue_avs(atp, pend, na):
        a0, a1, off0, off1, len1, pts = pend
        for hh in range(2):
            nc.tensor.matmul(
                atp[hh][:, off0:], lhsT=v_sb[:, a0, :],
                rhs=pts[hh][:, off0:BN], start=(a0 == 0), stop=False)
            nc.tensor.matmul(
                atp[hh][:, off1:], lhsT=v_sb[:, a1, :],
                rhs=pts[hh][:, BN:BN + len1], start=False, stop=(a1 == na - 1))

    # ---------------- schedule ----------------
    for f in proj_fillers(0):
        f()
    att_block(0, proj_fillers(1))
    # prefetch full WoT during the attention phase (gpsimd DMA queue; the
    # only other things on gpsimd are the collective triggers and ork loads)
    _wosrc = woT.rearrange("(a p) n -> p a n", p=P)
    for c in range(DT):
        nc.gpsimd.dma_start(out=woT_sb[:, c:c + 1, :], in_=_wosrc[:, c:c + 1, :])
    att_block(1, proj_fillers(2))
    trigger_a2a(a2a_in0, a2a_out0)
    att_block(2, proj_fillers(3))
    # ork0 loads on the sync queue. A dma_start blocks its issuing engine
    # until the source is ready, and a BLOCKED GPSIMD stalls the CC stream
    # (A2As measured 2x slower with ork loads parked on gpsimd). On sync the
    # only ordering requirement is that A2A#0 completes before block 3's
    # pair-0 normalize needs the queue -- which it does with a wide margin.
    ork0 = []
    a2a0_r = a2a_out0.rearrange("(a p) m -> p a m", p=P)
    for k in range(DT):
        rt = orhs.tile([P, TOK], BF16, tag="rt", name=f"rt0_{k}")
        nc.sync.dma_start(out=rt, in_=a2a0_r[:, k, :])
        ork0.append(rt)
    # ork1: even k-tiles come from the pair-0 half A2A, odd from pair-1.
    # BOTH pairs' loads are emitted only at pair-1's end, after every
    # normalize a2a_in write is already queued on sync.
    ork1 = [None] * DT

    def b3_pair_end(pair):
        trigger_a2a(a2a_in1[pair], a2a_out1[pair])
        if pair == 0:
            return
        for p in range(2):
            a2a1_r = a2a_out1[p].rearrange("(a p) m -> p a m", p=P)
            for s in range(NCORES):
                rt = orhs.tile([P, TOK], BF16, tag="rt", name=f"rt1_{2 * s + p}")
                nc.sync.dma_start(out=rt, in_=a2a1_r[:, s, :])
                ork1[2 * s + p] = rt

    att_block(3, [lambda: oproj_chunk(0, 0, ork0), lambda: oproj_chunk(0, 1, ork0)],
              pair_end=b3_pair_end)
    # tail: even-k g1 groups (data from A2A#1a) interleave ACROSS od blocks
    # while A2A#1b is in flight; only odd-k accumulations wait on it. ps_b has
    # 2 bufs -> at most two open o_proj-1 psum groups at a time.
    evens = [0, 2, 4, 6, 8, 10, 12, 14]
    odds = [1, 3, 5, 7, 9, 11, 13, 15]
    oproj_chunk(0, 2, ork0)
    pso0 = oproj_chunk(1, 0, ork1, korder=evens, finish=False)
    oproj_chunk(0, 3, ork0)
    pso1 = oproj_chunk(1, 1, ork1, korder=evens, finish=False)
    if taps is not None:
        nc.sync.dma_start(out=taps["qT_d"][:, :, :], in_=qT_sb)
        nc.sync.dma_start(out=taps["kT_d"][:, :], in_=kT_sb)
        nc.sync.dma_start(out=taps["v_d"][:, :, :], in_=v_sb)
        nc.sync.dma_start(out=taps["a2a_in0_d"][:, :], in_=a2a_in0[:, :])
    oproj_chunk(1, 0, ork1, korder=odds, pso=pso0)
    pso2 = oproj_chunk(1, 2, ork1, korder=evens, finish=False)
    oproj_chunk(1, 1, ork1, korder=odds, pso=pso1)
    pso3 = oproj_chunk(1, 3, ork1, korder=evens, finish=False)
    oproj_chunk(1, 2, ork1, korder=odds, pso=pso2)
    oproj_chunk(1, 3, ork1, korder=odds, pso=pso3)


# ---------------- host side ----------------

def prep_in_maps(x, Wq, Wk, Wv, Wo):
    bf = ml_dtypes.bfloat16
    xTh = np.ascontiguousarray(x.reshape(S, D).T.astype(np.float16))
    trineg = np.where(np.arange(P)[:, None] <= np.arange(P)[None, :],
                      np.float16(0.0), np.float16(NMASK))
    trineg = np.ascontiguousarray(trineg.astype(np.float16))
    woT_h = np.ascontiguousarray(Wo.T.astype(bf))
    in_maps = []
    for c in range(NCORES):
        wq = Wq[c * JQ:(c + 1) * JQ, :].T
        wk = Wk[c * HD:(c + 1) * HD, :].T * SM
        wv = Wv[c * HD:(c + 1) * HD, :].T
        wqkv_h = np.ascontiguousarray(
            np.concatenate([wq, wv, wk], axis=1).astype(np.float16))
        in_maps.append({"xT": xTh, "wqkv": wqkv_h, "woT": woT_h,
                        "trineg": trineg})
    return in_maps


def unshard(results):
    out = np.empty((S, D), dtype=np.float32)
    for c in range(NCORES):
        o = np.asarray(results[c]["out"]).reshape(2, TOK, D)
        out[c * TOK:(c + 1) * TOK, :] = o[0]
        out[S // 2 + c * TOK:S // 2 + (c + 1) * TOK, :] = o[1]
    return out.reshape(1, S, D)


def kernel(x, Wq, Wk, Wv, Wo):
    from concourse.bass_utils import run_bass_kernel_spmd
    nc = build_nc()
    in_maps = prep_in_maps(x, Wq, Wk, Wv, Wo)
    res = run_bass_kernel_spmd(nc, in_maps, core_ids=list(range(NCORES)))
    return unshard(res.results)
